# revision 13
# baseline (speedup 1.0000x reference)
"""GroupAttention sparse-attention kernel for 8 trn2 NeuronCores.

Math (derived + numerically verified against the reference):
  - The mask keeps only tridiagonal scores -> softmax rows have >=1 finite
    entries at j=i+-1, or are fully uniform 1/S ("caseB" rows, where
    eos[i-1]=eos[i+1]=0).
  - neibor = v0 + (vBB-v0)*u u^T  (rank-1 over caseB flags u), overwritten on
    the 3 band diagonals with d_sup/d_main.
  - g[i,j] = exp(cum[j]-cum[i]) for j>i (sym.), diag d_main, +1e-9 off-diag,
    where cum = prefix-sum of ell=log(d_sup+1e-9).
  - scores use A~ = wq^T wk:  s[i,j] = xn_i A~ xn_j^T / 512.
SPMD: one program "compute rows 0..1023". core 2b -> batch b as-is;
core 2b+1 -> batch b with rows reversed (problem is reversal-covariant),
host un-reverses its output half. bq/bk/beta are zeros and gamma ones per the
problem spec, so they are folded away.
"""

import numpy as np
from contextlib import ExitStack

B, S, D = 4, 2048, 1024
NT = 8          # 128-row blocks per core (half of S/128)
HALF = S // 2

_cache = {}


def _build():
    import concourse.bass as bass
    import concourse.bacc as bacc
    import concourse.mybir as mybir
    from concourse.tile import TileContext

    f32 = mybir.dt.float32
    bf16 = mybir.dt.bfloat16
    i32 = mybir.dt.int32
    AF = mybir.ActivationFunctionType
    OP = mybir.AluOpType

    nc = bacc.Bacc("TRN2", target_bir_lowering=False)

    # ---------------- I/O ----------------
    x_in = nc.dram_tensor("x", [S, D], f32, kind="ExternalInput")
    eospad = nc.dram_tensor("eospad", [S + 2], i32, kind="ExternalInput")
    prior_t = nc.dram_tensor("prior", [1], f32, kind="ExternalInput")
    wq_in = nc.dram_tensor("wq", [D, D], f32, kind="ExternalInput")
    wk_in = nc.dram_tensor("wk", [D, D], f32, kind="ExternalInput")
    lt_in = nc.dram_tensor("lt128", [128, 128], f32, kind="ExternalInput")
    wup_in = nc.dram_tensor("wup", [128, 128], f32, kind="ExternalInput")
    wlo_in = nc.dram_tensor("wlo", [128, 128], f32, kind="ExternalInput")
    ones_in = nc.dram_tensor("onesb", [128, 1], bf16, kind="ExternalInput")
    zeros_in = nc.dram_tensor("zerosf", [16], f32, kind="ExternalInput")
    out_nb = nc.dram_tensor("out_nb", [HALF, S], f32, kind="ExternalOutput")
    out_g = nc.dram_tensor("out_g", [HALF, S], f32, kind="ExternalOutput")

    C_SQ9 = float(np.sqrt(np.float32(1e-9)))                    # sqrt(1e-9)
    C_SBB = float(np.sqrt(np.float32((1.0 / S) ** 2 + 1e-9)))   # caseB diag sqrt

    with TileContext(nc) as tc, ExitStack() as ctx:
        # ---------------- pools (whole-kernel lifetime) ----------------
        consts = ctx.enter_context(tc.tile_pool(name="consts", bufs=1))
        vec = ctx.enter_context(tc.tile_pool(name="vec", bufs=28))
        col = ctx.enter_context(tc.tile_pool(name="col", bufs=10))
        at_pool = ctx.enter_context(tc.tile_pool(name="atp", bufs=1))
        xnt_pool = ctx.enter_context(tc.tile_pool(name="xntp", bufs=1))
        psA = ctx.enter_context(tc.tile_pool(name="psA", bufs=2, space="PSUM"))
        psB = ctx.enter_context(tc.tile_pool(name="psB", bufs=1, space="PSUM"))
        dram = ctx.enter_context(tc.tile_pool(name="dram", bufs=1, space="DRAM"))

        # ---------------- consts into SBUF ----------------
        lt128 = consts.tile([128, 128], f32)
        nc.sync.dma_start(out=lt128, in_=lt_in[:, :])
        wup = consts.tile([128, 128], f32)
        nc.sync.dma_start(out=wup, in_=wup_in[:, :])
        wlo = consts.tile([128, 128], f32)
        nc.sync.dma_start(out=wlo, in_=wlo_in[:, :])
        ones_b = consts.tile([128, 1], bf16)
        nc.sync.dma_start(out=ones_b, in_=ones_in[:, :])
        pr_col = consts.tile([128, 1], f32)
        nc.sync.dma_start(
            out=pr_col,
            in_=bass.AP(tensor=prior_t[:].tensor, offset=prior_t[:].offset, ap=[[0, 128], [1, 1]]),
        )
        omp_col = consts.tile([128, 1], f32)  # 1 - prior
        nc.vector.tensor_scalar(omp_col, pr_col, -1.0, 1.0, OP.mult, OP.add)
        # v0 / vBB / (vBB-v0) as [128,1] broadcast columns
        v0_col = consts.tile([128, 1], f32)
        nc.vector.tensor_scalar(v0_col, omp_col, C_SQ9, None, OP.mult)
        nc.vector.tensor_tensor(v0_col, v0_col, pr_col, OP.add)
        vbb_col = consts.tile([128, 1], f32)
        nc.vector.tensor_scalar(vbb_col, omp_col, C_SBB, None, OP.mult)
        nc.vector.tensor_tensor(vbb_col, vbb_col, pr_col, OP.add)
        dv_col = consts.tile([128, 1], f32)  # vBB - v0
        nc.vector.tensor_tensor(dv_col, vbb_col, v0_col, OP.subtract)
        neg9 = consts.tile([128, 16], f32)
        nc.vector.memset(neg9, -1.0e9)
        # register const bias columns used by activation(bias=float)
        for ci, cval in enumerate((0.0, 1e-9, 1e-5)):
            cc = consts.tile([128, 1], f32, name=f"cc{ci}", tag=f"cc{ci}")
            nc.vector.memset(cc, cval)
            nc.const_aps.aps[(f32, cval)] = cc[:, :]

        # ---------------- DRAM scratch ----------------
        xb_d = dram.tile([S, D], bf16)          # normalized x, bf16
        snext_d = dram.tile([S], f32)
        sprev_d = dram.tile([S], f32)
        cum_d = dram.tile([S], f32)
        uscl_d = dram.tile([S], f32)            # (vBB-v0)*u
        u_d = dram.tile([S], f32)
        dsup_d = dram.tile([S + 1], f32)        # [0]=0, [1+i]=d_sup[i]
        dmain_d = dram.tile([S], f32)

        # ============ phase 1: LN+cast x ; A~^T = wk^T wq (bf16) ============
        with ExitStack() as p1:
            wpool = p1.enter_context(tc.tile_pool(name="wpool", bufs=1))
            xpool = p1.enter_context(tc.tile_pool(name="xpool", bufs=3))
            xbpool = p1.enter_context(tc.tile_pool(name="xbpool", bufs=3))
            stpool = p1.enter_context(tc.tile_pool(name="stpool", bufs=4))

            wqb = wpool.tile([128, 8, D], bf16)
            nc.gpsimd.dma_start(
                out=wqb[:, :, :], in_=wq_in[:, :].rearrange("(t p) e -> p t e", p=128)
            )
            wkb = wpool.tile([128, 8, D], bf16)
            nc.gpsimd.dma_start(
                out=wkb[:, :, :], in_=wk_in[:, :].rearrange("(t p) e -> p t e", p=128)
            )

            at_sb = at_pool.tile([128, 8, D], bf16)  # AT[p,ft,e] = A~^T[f,e]
            for ft in range(8):
                ps = psA.tile([128, D], f32)
                for dt in range(8):
                    for c in range(2):
                        nc.tensor.matmul(
                            ps[:, c * 512:(c + 1) * 512],
                            wkb[:, dt, ft * 128:(ft + 1) * 128],
                            wqb[:, dt, c * 512:(c + 1) * 512],
                            start=(dt == 0),
                            stop=(dt == 7),
                        )
                if ft % 2 == 0:
                    nc.vector.tensor_copy(out=at_sb[:, ft, :], in_=ps[:, :])
                else:
                    nc.scalar.copy(out=at_sb[:, ft, :], in_=ps[:, :])

            # --- LN per 128-row tile, write bf16 normalized x to DRAM ---
            for it in range(16):
                xt = xpool.tile([128, D], f32)
                nc.sync.dma_start(out=xt, in_=x_in[it * 128:(it + 1) * 128, :])
                stats = stpool.tile([128, 2, 6], f32)
                nc.vector.bn_stats(out=stats[:, 0, :], in_=xt[:, 0:512])
                nc.vector.bn_stats(out=stats[:, 1, :], in_=xt[:, 512:1024])
                mv = stpool.tile([128, 2], f32)
                nc.vector.bn_aggr(out=mv, in_=stats)
                # rstd = exp(-0.5*ln(var+1e-5))
                rstd = stpool.tile([128, 1], f32)
                nc.scalar.activation(rstd, mv[:, 1:2], AF.Ln, bias=1e-5)
                nc.scalar.activation(rstd, rstd, AF.Exp, scale=-0.5)
                xbt = xbpool.tile([128, D], bf16)
                nc.vector.tensor_scalar(
                    xbt, xt, mv[:, 0:1], rstd, OP.subtract, OP.mult
                )
                nc.sync.dma_start(out=xb_d[it * 128:(it + 1) * 128, :], in_=xbt)

        # ============ phase 2: transpose; z; band dot-products ============
        xnt = xnt_pool.tile([128, 8, S], bf16)   # xnt[p,ft,i] = xn[i, ft*128+p]
        for ft in range(8):
            nc.sync.dma_start(
                out=xnt[:, ft, :], in_=xb_d[:, ft * 128:(ft + 1) * 128],
                transpose=True,
            )

        with ExitStack() as p2:
            zpool = p2.enter_context(tc.tile_pool(name="zpool", bufs=2))
            p1pool = p2.enter_context(tc.tile_pool(name="p1pool", bufs=2))
            p2pool = p2.enter_context(tc.tile_pool(name="p2pool", bufs=8))
            rows = p2.enter_context(tc.tile_pool(name="rows", bufs=2))

            ps_n = psB.tile([1, S], f32, tag="psrow", name="ps_n")          # s_next accumulator
            p2tiles = []
            for et in range(8):
                zb = zpool.tile([128, S], bf16)
                for half in range(2):
                    ps = psA.tile([128, 1024], f32)
                    for ft in range(8):
                        for c in range(2):
                            off = half * 1024 + c * 512
                            nc.tensor.matmul(
                                ps[:, c * 512:(c + 1) * 512],
                                at_sb[:, ft, et * 128:(et + 1) * 128],
                                xnt[:, ft, off:off + 512],
                                start=(ft == 0),
                                stop=(ft == 7),
                            )
                    eng_copy = nc.vector.tensor_copy if half == 0 else nc.scalar.copy
                    eng_copy(out=zb[:, half * 1024:(half + 1) * 1024], in_=ps)
                # products
                pt1 = p1pool.tile([128, S], bf16)
                nc.vector.tensor_tensor(
                    pt1[:, 0:S - 1], xnt[:, et, 0:S - 1], zb[:, 1:S], OP.mult
                )
                pt2 = p2pool.tile([128, S], bf16)
                nc.vector.tensor_tensor(
                    pt2[:, 1:S], xnt[:, et, 1:S], zb[:, 0:S - 1], OP.mult
                )
                p2tiles.append(pt2)
                for c in range(4):
                    nc.tensor.matmul(
                        ps_n[0:1, c * 512:(c + 1) * 512],
                        ones_b,
                        pt1[:, c * 512:(c + 1) * 512],
                        start=(et == 0),
                        stop=(et == 7),
                    )
            row_n = rows.tile([1, S], f32)
            nc.scalar.mul(row_n, ps_n[0:1, :], 1.0 / 512.0)
            nc.sync.dma_start(out=snext_d[:], in_=row_n)

            ps_p = psB.tile([1, S], f32, tag="psrow", name="ps_p")
            for et in range(8):
                for c in range(4):
                    nc.tensor.matmul(
                        ps_p[0:1, c * 512:(c + 1) * 512],
                        ones_b,
                        p2tiles[et][:, c * 512:(c + 1) * 512],
                        start=(et == 0),
                        stop=(et == 7),
                    )
            row_p = rows.tile([1, S], f32)
            nc.scalar.mul(row_p, ps_p[0:1, :], 1.0 / 512.0)
            nc.sync.dma_start(out=sprev_d[:], in_=row_p)

        # ============ phase 3: band math in [128,16] layout ============
        def v16():
            return vec.tile([128, 16], f32, tag="v16", name="v16")

        def rd16(dtensor, off):  # dram vec [off:off+2048] -> [128,16] row-major
            return dtensor[off:off + S].rearrange("(p c) -> p c", c=16)

        sn = v16()
        nc.sync.dma_start(out=sn, in_=rd16(snext_d, 0))
        sp = v16()
        nc.sync.dma_start(out=sp, in_=rd16(sprev_d, 0))
        em_i = vec.tile([128, 16], i32)
        nc.sync.dma_start(out=em_i, in_=rd16(eospad[:], 1))
        hn_i = vec.tile([128, 16], i32)
        nc.sync.dma_start(out=hn_i, in_=rd16(eospad[:], 2))
        hp_i = vec.tile([128, 16], i32)
        nc.sync.dma_start(out=hp_i, in_=rd16(eospad[:], 0))
        hn = v16()
        nc.vector.tensor_copy(out=hn, in_=hn_i)
        hp = v16()
        nc.vector.tensor_copy(out=hp, in_=hp_i)

        sne = v16()
        nc.vector.select(sne, hn_i, sn, neg9)
        spe = v16()
        nc.vector.select(spe, hp_i, sp, neg9)
        m = v16()
        nc.vector.tensor_tensor(m, sne, spe, OP.max)
        en = v16()
        nc.vector.tensor_tensor(en, sne, m, OP.subtract)
        nc.scalar.activation(en, en, AF.Exp)
        ep = v16()
        nc.vector.tensor_tensor(ep, spe, m, OP.subtract)
        nc.scalar.activation(ep, ep, AF.Exp)
        zs = v16()
        nc.vector.tensor_tensor(zs, en, ep, OP.add)
        rz = v16()
        nc.vector.reciprocal(rz, zs)
        nn = v16()
        nc.vector.tensor_tensor(nn, en, rz, OP.mult)
        npv = v16()
        nc.vector.tensor_tensor(npv, ep, rz, OP.mult)
        # caseB flag u = (1-hn)*(1-hp); blend N with uniform 1/S
        t1 = v16()
        nc.vector.tensor_scalar(t1, hn, -1.0, 1.0, OP.mult, OP.add)
        t2 = v16()
        nc.vector.tensor_scalar(t2, hp, -1.0, 1.0, OP.mult, OP.add)
        cb = v16()
        nc.vector.tensor_tensor(cb, t1, t2, OP.mult)
        omcb = v16()
        nc.vector.tensor_scalar(omcb, cb, -1.0, 1.0, OP.mult, OP.add)
        cbS = v16()
        nc.vector.tensor_scalar(cbS, cb, 1.0 / S, None, OP.mult)
        for nv in (nn, npv):
            nc.vector.tensor_tensor(nv, nv, omcb, OP.mult)
            nc.vector.tensor_tensor(nv, nv, cbS, OP.add)
        # Np shifted by +1 (value at i+1)
        npsh = v16()
        nc.vector.memset(npsh, 0.0)
        nc.vector.tensor_copy(out=npsh[:, 0:15], in_=npv[:, 1:16])
        nc.sync.dma_start(out=npsh[0:127, 15:16], in_=npv[1:128, 0:1])
        msup = v16()
        nc.vector.tensor_tensor(msup, nn, npsh, OP.mult)
        # d_sup = prior + (1-prior)*exp(0.5*ln(msup+1e-9))
        dsup = v16()
        nc.scalar.activation(dsup, msup, AF.Ln, bias=1e-9)
        nc.scalar.activation(dsup, dsup, AF.Exp, scale=0.5)
        nc.vector.tensor_scalar(dsup, dsup, omp_col, pr_col, OP.mult, OP.add)
        # d_main = prior + (1-prior)*(c1 + (c2-c1)*cb)
        dmain = v16()
        nc.vector.tensor_scalar(dmain, cb, C_SBB - C_SQ9, C_SQ9, OP.mult, OP.add)
        nc.vector.tensor_scalar(dmain, dmain, omp_col, pr_col, OP.mult, OP.add)
        # ell, prefix sums
        ell = v16()
        nc.scalar.activation(ell, dsup, AF.Ln, bias=1e-9)
        zv16 = v16()
        nc.vector.memset(zv16, 0.0)
        incl = v16()
        nc.vector.tensor_tensor_scan(incl, ell, zv16, 0.0, OP.add, OP.add)
        excl = v16()
        nc.vector.tensor_tensor(excl, incl, ell, OP.subtract)
        ps_c = psA.tile([128, 1024], f32, tag="ps", name="ps_c")
        nc.tensor.matmul(
            ps_c[:, 0:1], lt128, incl[:, 15:16], start=True, stop=True
        )
        cp_col = col.tile([128, 1], f32)
        nc.vector.tensor_copy(out=cp_col, in_=ps_c[:, 0:1])
        cum = v16()
        nc.vector.tensor_scalar(cum, excl, cp_col, None, OP.add)
        uscl = v16()
        nc.vector.tensor_scalar(uscl, cb, dv_col, None, OP.mult)

        def wr16(dtensor, off, src):
            nc.sync.dma_start(
                out=dtensor[off:off + S].rearrange("(p c) -> p c", c=16), in_=src
            )

        wr16(cum_d, 0, cum)
        wr16(uscl_d, 0, uscl)
        wr16(u_d, 0, cb)
        wr16(dsup_d, 1, dsup)
        wr16(dmain_d, 0, dmain)

        # ============ phase 4: outputs ============
        with ExitStack() as p3:
            bcast = p3.enter_context(tc.tile_pool(name="bcast", bufs=1))
            outp = p3.enter_context(tc.tile_pool(name="outp", bufs=3))
            gwin = p3.enter_context(tc.tile_pool(name="gwin", bufs=6))
            colp = p3.enter_context(tc.tile_pool(name="colp", bufs=1))

            urow = bcast.tile([128, S], f32)
            nc.sync.dma_start(
                out=urow,
                in_=bass.AP(tensor=uscl_d[:].tensor, offset=uscl_d[:].offset,
                            ap=[[0, 128], [1, S]]),
            )
            cumrow = bcast.tile([128, S], f32)
            nc.sync.dma_start(
                out=cumrow,
                in_=bass.AP(tensor=cum_d[:].tensor, offset=cum_d[:].offset,
                            ap=[[0, 128], [1, S]]),
            )
            ucols = colp.tile([128, 8], f32)
            nc.sync.dma_start(
                out=ucols, in_=u_d[0:HALF].rearrange("(t p) -> p t", p=128)
            )
            cumcols = colp.tile([128, 8], f32)
            nc.sync.dma_start(
                out=cumcols, in_=cum_d[0:HALF].rearrange("(t p) -> p t", p=128)
            )

            for t in range(NT):
                r0 = t * 128
                nb = outp.tile([128, S], f32)
                nc.vector.tensor_scalar(
                    nb, urow, ucols[:, t:t + 1], v0_col, OP.mult, OP.add
                )
                nc.sync.dma_start(out=out_nb[r0:r0 + 128, :], in_=nb)

                g = outp.tile([128, S], f32)
                nc.vector.tensor_scalar(
                    g, cumrow, cumcols[:, t:t + 1], None, OP.subtract
                )
                if t > 0:
                    nc.scalar.activation(g[:, 0:r0], g[:, 0:r0], AF.Exp, scale=-1.0)
                nc.scalar.activation(
                    g[:, r0 + 128:S], g[:, r0 + 128:S], AF.Exp, scale=1.0
                )
                w = g[:, r0:r0 + 128]
                c1t = gwin.tile([128, 128], f32)
                nc.vector.tensor_scalar(c1t, w, 0.5, None, OP.min)
                e1 = gwin.tile([128, 128], f32)
                nc.scalar.activation(e1, c1t, AF.Exp)
                c2t = gwin.tile([128, 128], f32)
                nc.vector.tensor_scalar(c2t, w, -0.5, None, OP.max)
                e2 = gwin.tile([128, 128], f32)
                nc.scalar.activation(e2, c2t, AF.Exp, scale=-1.0)
                nc.vector.tensor_tensor(e1, e1, wup, OP.mult)
                nc.vector.tensor_tensor(e2, e2, wlo, OP.mult)
                nc.vector.tensor_tensor(w, e1, e2, OP.add)
                nc.gpsimd.tensor_scalar(g, g, 1.0e-9, None, OP.add)
                nc.sync.dma_start(out=out_g[r0:r0 + 128, :], in_=g)

            # band diagonals straight into DRAM (strided DRAM->DRAM copies)
            def diag_ap(dt, offset, count):
                return bass.AP(tensor=dt[:, :].tensor, offset=dt[:, :].offset + offset,
                               ap=[[S + 1, count]])

            nc.sync.dma_start(out=diag_ap(out_nb, 1, HALF), in_=dsup_d[1:1 + HALF])
            nc.sync.dma_start(out=diag_ap(out_nb, S, HALF - 1),
                              in_=dsup_d[1:HALF])
            nc.sync.dma_start(out=diag_ap(out_nb, 0, HALF), in_=dmain_d[0:HALF])
            nc.sync.dma_start(out=diag_ap(out_g, 0, HALF), in_=dmain_d[0:HALF])

    nc.compile()
    return nc


def _consts():
    k = np.arange(128)
    lt = (k[:, None] < k[None, :]).astype(np.float32)       # lt[k,p]=k<p
    wup = (k[None, :] > k[:, None]).astype(np.float32)      # wup[p,w]=w>p
    wlo = (k[None, :] < k[:, None]).astype(np.float32)
    import ml_dtypes
    ones = np.ones((128, 1), dtype=ml_dtypes.bfloat16)
    zer = np.zeros(16, np.float32)
    return lt, wup, wlo, ones, zer


def kernel(context, eos_mask, prior, wq, bq, wk, bk, gamma, beta):
    from concourse.bass_utils import run_bass_kernel_spmd

    if "nc" not in _cache:
        _cache["nc"] = _build()
    nc = _cache["nc"]

    context = np.asarray(context, np.float32)
    eos_mask = np.asarray(eos_mask, np.int32)
    prior = np.asarray(prior, np.float32)
    wq = np.asarray(wq, np.float32)
    wk = np.asarray(wk, np.float32)
    lt, wup, wlo, ones, zer = _consts()

    in_maps = []
    for c in range(8):
        b, h = c // 2, c % 2
        x = context[b] if h == 0 else context[b][::-1]
        eo = eos_mask[b] if h == 0 else eos_mask[b][::-1]
        eop = np.zeros(S + 2, np.int32)
        eop[1:S + 1] = eo
        in_maps.append({
            "x": np.ascontiguousarray(x),
            "eospad": eop,
            "prior": prior,
            "wq": wq, "wk": wk,
            "lt128": lt, "wup": wup, "wlo": wlo,
            "onesb": ones, "zerosf": zer,
        })

    bkr = run_bass_kernel_spmd(nc, in_maps, core_ids=list(range(8)))
    _cache["last_bkr"] = bkr

    g_out = np.empty((B, S, S), np.float32)
    nb_out = np.empty((B, S, S), np.float32)
    for c in range(8):
        b, h = c // 2, c % 2
        rg = bkr.results[c]["out_g"]
        rn = bkr.results[c]["out_nb"]
        if h == 0:
            g_out[b, :HALF] = rg
            nb_out[b, :HALF] = rn
        else:
            g_out[b, HALF:] = rg[::-1, ::-1]
            nb_out[b, HALF:] = rn[::-1, ::-1]
    return g_out, nb_out
